# revision 19
# baseline (speedup 1.0000x reference)
"""LSTMCell (B=16384, I=H=512) on 8 Trainium2 NeuronCores.

Strategy: data-parallel over the batch (2048 rows/core). Each core computes
gatesT = W @ [x;h]T in transposed layout (gate dim on partitions, batch on the
free dim) so that:
  - the contraction dim (I+H) lands on SBUF partitions for both matmul
    operands with zero on-chip transposes (inputs are pre-transposed on the
    host while sharding),
  - the gate bias is a per-partition vector, applied for free by the ScalarE
    activation instruction (which also folds in the fp8 weight descale).
Mixed precision: the i and f gates run in fp8e4 with perf_mode=DoubleRow
(2 fp8 weights per PE cell, contraction 256/instruction) which cuts their
matmul stream cycles in half; g and o stay bf16 (error analysis: fp8 error
on i+f lands at ~1.7e-2 rel_l2, within the 2e-2 budget, while touching g or
o would exceed it). Weights for the fp8 gates are host-scaled by 64 to avoid
the e4m3 denormal range; ScalarE activation applies scale=1/64.
All wire tensors are bf16/fp8: ~12MB HBM traffic per core vs 28MB for f32.
Elementwise LSTM tail (sigmoid/tanh/mul/add) runs on ScalarE + VectorE
overlapped with the matmuls; outputs are stored transposed in bf16 and
un-transposed/upcast on the host.
"""

import numpy as np
import ml_dtypes
from contextlib import ExitStack

_B, _I, _H = 16384, 512, 512
_NC = 8
_BL = _B // _NC          # 2048 batch rows per core
_G = 4 * _H              # 2048 stacked gate dim
_K = _I + _H             # 1024 contraction dim
_BCH = 512               # batch chunk (PSUM bank free size)
_NB = _BL // _BCH        # 4 batch chunks
_NJ = _H // 128          # 4 h-blocks of 128
_NK = _K // 128          # 8 k-chunks of 128
_NQ = _NK // 2           # 4 DoubleRow k-pairs
_W8SCALE = 64.0

_cache = {}


def _build(reps=1):
    from concourse import bacc
    import concourse.mybir as mybir
    import concourse.tile as tile

    f32 = mybir.dt.float32
    bf16 = mybir.dt.bfloat16
    f8 = mybir.dt.float8e4
    AF = mybir.ActivationFunctionType
    DR = mybir.MatmulPerfMode.DoubleRow

    nc = bacc.Bacc("TRN2", target_bir_lowering=False, debug=False,
                   num_devices=_NC)
    xT = nc.declare_dram_parameter("xT", [_I, _BL], bf16, isOutput=False)
    hT = nc.declare_dram_parameter("hT", [_H, _BL], bf16, isOutput=False)
    cT = nc.declare_dram_parameter("cT", [_H, _BL], bf16, isOutput=False)
    # bf16 weights for the g/o gates: row k, col j*256 + u*128 + m (u: g,o)
    wTb = nc.declare_dram_parameter("wTb", [_K, _G // 2], bf16, isOutput=False)
    # fp8 weights for the i/f gates, DoubleRow-interleaved: row q*128 + p,
    # col s*1024 + j*256 + t*128 + m holds W[t-gate, j, m, k=q*256+s*128+p]*64
    wT8 = nc.declare_dram_parameter("wT8", [_K // 2, _G], f8, isOutput=False)
    # fp8 activations, DoubleRow-interleaved: row q*128 + p,
    # col s*2048 + b holds xh[k=q*256+s*128+p, b]
    xh8d = nc.declare_dram_parameter("xh8", [_K // 2, 2 * _BL], f8,
                                     isOutput=False)
    b2 = nc.declare_dram_parameter("b2", [128, _G // 128], f32, isOutput=False)
    hoT = nc.declare_dram_parameter("hoT", [_H, _BL], bf16, isOutput=True)
    coT = nc.declare_dram_parameter("coT", [_H, _BL], bf16, isOutput=True)

    with ExitStack() as ctx:
        tc = ctx.enter_context(tile.TileContext(nc))
        wp = ctx.enter_context(tc.tile_pool(name="w", bufs=2))
        xp = ctx.enter_context(tc.tile_pool(name="xh", bufs=2))
        bp = ctx.enter_context(tc.tile_pool(name="bias", bufs=1))
        cp = ctx.enter_context(tc.tile_pool(name="cin", bufs=3))
        ap = ctx.enter_context(tc.tile_pool(name="act", bufs=2))
        op = ctx.enter_context(tc.tile_pool(name="out", bufs=2))
        pp = ctx.enter_context(tc.tile_pool(name="ps", bufs=2, space="PSUM"))

        def body(_iv=None):
            bias_sb = bp.tile([128, _G // 128], f32, tag="bias")
            nc.sync.dma_start(out=bias_sb[:], in_=b2[:])

            # Batched loads (all on the SP HWDGE queue; c/outs go on the
            # ScalarE queue inside the group loop). Every tile covers all
            # j/ch so the whole rep's working set streams in up front;
            # bufs=2 lets rep r+1's loads run behind rep r's compute.
            w8_sb, x8_sb, wb_sb, xb_sb = [], [], [], []
            for q in range(_NQ):
                t_ = wp.tile([128, 2, _G // 2], f8, tag=f"w8_{q}")
                nc.sync.dma_start(out=t_[:], in_=wT8[q * 128:(q + 1) * 128, :])
                w8_sb.append(t_)
            for q in range(_NQ):
                t_ = xp.tile([128, 2, _BL], f8, tag=f"x8_{q}")
                nc.sync.dma_start(out=t_[:],
                                  in_=xh8d[q * 128:(q + 1) * 128, :])
                x8_sb.append(t_)
            for k in range(_NK):
                t_ = wp.tile([128, _G // 2], bf16, tag=f"wb_{k}")
                nc.sync.dma_start(out=t_[:], in_=wTb[k * 128:(k + 1) * 128, :])
                wb_sb.append(t_)
            for k in range(_NK):
                t_ = xp.tile([128, _BL], bf16, tag=f"xb_{k}")
                src = xT if k < _NK // 2 else hT
                r = (k % (_NK // 2)) * 128
                nc.sync.dma_start(out=t_[:], in_=src[r:r + 128, :])
                xb_sb.append(t_)

            for ch in range(_NB):
                bsl = slice(ch * _BCH, (ch + 1) * _BCH)
                for j in range(_NJ):
                    c_sb = cp.tile([128, _BCH], bf16, tag="c")
                    nc.scalar.dma_start(out=c_sb[:],
                                        in_=cT[j * 128:(j + 1) * 128, bsl])
                    ps = []
                    for t in range(2):       # i, f gates: fp8 DoubleRow
                        pstile = pp.tile([128, _BCH], f32, tag=f"ps{t}")
                        col = j * 256 + t * 128
                        for q in range(_NQ):
                            nc.tensor.matmul(
                                pstile[:],
                                w8_sb[q][:, :, col:col + 128],
                                x8_sb[q][:, :, bsl],
                                start=(q == 0), stop=(q == _NQ - 1),
                                perf_mode=DR,
                            )
                        ps.append(pstile)
                    for t in range(2):       # g, o gates: bf16
                        pstile = pp.tile([128, _BCH], f32, tag=f"ps{2 + t}")
                        col = j * 256 + t * 128
                        for k in range(_NK):
                            nc.tensor.matmul(
                                pstile[:],
                                wb_sb[k][:, col:col + 128],
                                xb_sb[k][:, bsl],
                                start=(k == 0), stop=(k == _NK - 1),
                            )
                        ps.append(pstile)
                    gI = ap.tile([128, _BCH], f32, tag="gI")
                    gF = ap.tile([128, _BCH], f32, tag="gF")
                    gG = ap.tile([128, _BCH], f32, tag="gG")
                    gO = ap.tile([128, _BCH], f32, tag="gO")
                    bcol = j * 4
                    nc.scalar.activation(gI[:], ps[0][:], AF.Sigmoid,
                                         bias=bias_sb[:, bcol + 0:bcol + 1],
                                         scale=1.0 / _W8SCALE)
                    nc.scalar.activation(gF[:], ps[1][:], AF.Sigmoid,
                                         bias=bias_sb[:, bcol + 1:bcol + 2],
                                         scale=1.0 / _W8SCALE)
                    nc.scalar.activation(gG[:], ps[2][:], AF.Tanh,
                                         bias=bias_sb[:, bcol + 2:bcol + 3])
                    nc.scalar.activation(gO[:], ps[3][:], AF.Sigmoid,
                                         bias=bias_sb[:, bcol + 3:bcol + 4])
                    newc = op.tile([128, _BCH], bf16, tag="newc")
                    newh = op.tile([128, _BCH], bf16, tag="newh")
                    nc.vector.tensor_mul(gF[:], gF[:], c_sb[:])   # f * c
                    nc.vector.tensor_mul(gI[:], gI[:], gG[:])     # i * g
                    nc.vector.tensor_add(newc[:], gF[:], gI[:])
                    nc.scalar.activation(gG[:], newc[:], AF.Tanh)
                    nc.vector.tensor_mul(newh[:], gO[:], gG[:])
                    nc.scalar.dma_start(out=coT[j * 128:(j + 1) * 128, bsl],
                                        in_=newc[:])
                    nc.scalar.dma_start(out=hoT[j * 128:(j + 1) * 128, bsl],
                                        in_=newh[:])

        if reps == 1:
            body()
        elif reps < 0:           # python-unrolled (sim analysis only)
            for _ in range(-reps):
                body()
        else:
            with tc.For_i(0, reps, 1):
                body()
    nc.compile()
    return nc


_BF16 = ml_dtypes.bfloat16
_F8 = ml_dtypes.float8_e4m3


def _host_prep_weights(Wi, bi, Wh, bh):
    W = np.concatenate([np.asarray(Wi, np.float32),
                        np.asarray(Wh, np.float32)], axis=1)    # [G, K]
    Wr = W.reshape(4, _NJ, 128, _K)                              # [t, j, m, k]
    # bf16 g/o weights: [k, j*256 + u*128 + m]
    wTb = np.ascontiguousarray(
        Wr[2:].transpose(3, 1, 0, 2).reshape(_K, _G // 2)).astype(_BF16)
    # fp8 i/f weights, DoubleRow layout: [q*128+p, s*1024 + j*256 + t*128 + m]
    W8 = (Wr[:2] * _W8SCALE).astype(_F8)                         # [t, j, m, k]
    W8k = W8.reshape(2, _NJ, 128, _NQ, 2, 128)                   # [t,j,m,q,s,p]
    wT8 = np.ascontiguousarray(
        W8k.transpose(3, 5, 4, 1, 0, 2).reshape(_K // 2, _G))
    # bias, per-partition layout: col = j*4 + t
    b = np.asarray(bi, np.float32) + np.asarray(bh, np.float32)
    br = b.reshape(4, _NJ, 128)                                  # [t, j, p]
    b2 = np.ascontiguousarray(br.transpose(2, 1, 0).reshape(128, _G // 128))
    return wTb, wT8, b2


def _host_shards(x, h, c, Wi, bi, Wh, bh):
    wTb, wT8, b2 = _host_prep_weights(Wi, bi, Wh, bh)
    xf = np.asarray(x, np.float32)
    hf = np.asarray(h, np.float32)
    cf = np.asarray(c, np.float32)
    in_maps = []
    for s in range(_NC):
        sl = slice(s * _BL, (s + 1) * _BL)
        xTs = np.ascontiguousarray(xf[sl].T)                     # [I, BL]
        hTs = np.ascontiguousarray(hf[sl].T)                     # [H, BL]
        xhT = np.concatenate([xTs, hTs], axis=0)                 # [K, BL]
        x8 = xhT.astype(_F8).reshape(_NQ, 2, 128, _BL)           # [q, s, p, b]
        xh8 = np.ascontiguousarray(
            x8.transpose(0, 2, 1, 3).reshape(_K // 2, 2 * _BL))
        in_maps.append({
            "xT": xTs.astype(_BF16),
            "hT": hTs.astype(_BF16),
            "cT": np.ascontiguousarray(cf[sl].T).astype(_BF16),
            "wTb": wTb,
            "wT8": wT8,
            "xh8": xh8,
            "b2": b2,
        })
    return in_maps


def kernel(x, h, c, Wi, bi, Wh, bh):
    from concourse.bass_utils import run_bass_kernel_spmd

    nc = _cache.get("nc")
    if nc is None:
        nc = _build()
        _cache["nc"] = nc

    in_maps = _host_shards(x, h, c, Wi, bi, Wh, bh)
    res = run_bass_kernel_spmd(nc, in_maps, list(range(_NC)))

    h_out = np.empty((_B, _H), np.float32)
    c_out = np.empty((_B, _H), np.float32)
    for s in range(_NC):
        sl = slice(s * _BL, (s + 1) * _BL)
        h_out[sl] = res.results[s]["hoT"].T.astype(np.float32)
        c_out[sl] = res.results[s]["coT"].T.astype(np.float32)
    return h_out, c_out


# revision 25
# speedup vs baseline: 2.1129x; 2.1129x over previous
"""LSTMCell (B=16384, I=H=512) on 8 Trainium2 NeuronCores.

Strategy: data-parallel over the batch (2048 rows/core). Each core computes
gatesT = W @ [x;h]T in transposed layout (gate dim on partitions, batch on the
free dim) so that:
  - the contraction dim (I+H) lands on SBUF partitions for both matmul
    operands with zero on-chip transposes (inputs are pre-transposed on the
    host while sharding),
  - the gate bias is a per-partition vector, applied for free by the ScalarE
    activation instruction (which also folds in the fp8 weight descale).
Mixed precision: the i and f gates run in fp8e4 with perf_mode=DoubleRow
(2 fp8 weights per PE cell, contraction 256/instruction) which cuts their
matmul stream cycles in half; g and o stay bf16 (error analysis: fp8 error
on i+f lands at ~1.7e-2 rel_l2, within the 2e-2 budget, while touching g or
o would exceed it). Weights for the fp8 gates are host-scaled by 64 to avoid
the e4m3 denormal range; ScalarE activation applies scale=1/64.
All wire tensors are bf16/fp8: ~12MB HBM traffic per core vs 28MB for f32.
Elementwise LSTM tail (sigmoid/tanh/mul/add) runs on ScalarE + VectorE
overlapped with the matmuls; outputs are stored transposed in bf16 and
un-transposed/upcast on the host.
"""

import numpy as np
import ml_dtypes
from contextlib import ExitStack

_B, _I, _H = 16384, 512, 512
_NC = 8
_BL = _B // _NC          # 2048 batch rows per core
_G = 4 * _H              # 2048 stacked gate dim
_K = _I + _H             # 1024 contraction dim
_BCH = 512               # batch chunk (PSUM bank free size)
_NB = _BL // _BCH        # 4 batch chunks
_NJ = _H // 128          # 4 h-blocks of 128
_NK = _K // 128          # 8 k-chunks of 128
_NQ = _NK // 2           # 4 DoubleRow k-pairs
_W8SCALE = 64.0

_cache = {}


def _build(reps=1):
    from concourse import bacc
    import concourse.mybir as mybir
    import concourse.tile as tile

    f32 = mybir.dt.float32
    bf16 = mybir.dt.bfloat16
    f8 = mybir.dt.float8e4
    AF = mybir.ActivationFunctionType
    DR = mybir.MatmulPerfMode.DoubleRow

    nc = bacc.Bacc("TRN2", target_bir_lowering=False, debug=False,
                   num_devices=_NC)
    xT = nc.declare_dram_parameter("xT", [_I, _BL], bf16, isOutput=False)
    hT = nc.declare_dram_parameter("hT", [_H, _BL], bf16, isOutput=False)
    cT = nc.declare_dram_parameter("cT", [_H, _BL], bf16, isOutput=False)
    # bf16 weights for the g/o gates: row k, col j*256 + u*128 + m (u: g,o)
    wTb = nc.declare_dram_parameter("wTb", [_K, _G // 2], bf16, isOutput=False)
    # fp8 weights for the i/f gates, DoubleRow-interleaved: row q*128 + p,
    # col s*1024 + j*256 + t*128 + m holds W[t-gate, j, m, k=q*256+s*128+p]*64
    wT8 = nc.declare_dram_parameter("wT8", [_K // 2, _G], f8, isOutput=False)
    # fp8 activations, DoubleRow-interleaved: row q*128 + p,
    # col s*2048 + b holds xh[k=q*256+s*128+p, b]
    xh8d = nc.declare_dram_parameter("xh8", [_K // 2, 2 * _BL], f8,
                                     isOutput=False)
    b2 = nc.declare_dram_parameter("b2", [128, _G // 128], f32, isOutput=False)
    hoT = nc.declare_dram_parameter("hoT", [_H, _BL], bf16, isOutput=True)
    coT = nc.declare_dram_parameter("coT", [_H, _BL], bf16, isOutput=True)

    with ExitStack() as ctx:
        tc = ctx.enter_context(tile.TileContext(nc))
        wp = ctx.enter_context(tc.tile_pool(name="w", bufs=2))
        xp = ctx.enter_context(tc.tile_pool(name="xh", bufs=2))
        bp = ctx.enter_context(tc.tile_pool(name="bias", bufs=1))
        cp = ctx.enter_context(tc.tile_pool(name="cin", bufs=3))
        ap = ctx.enter_context(tc.tile_pool(name="act", bufs=2))
        op = ctx.enter_context(tc.tile_pool(name="out", bufs=2))
        pp = ctx.enter_context(tc.tile_pool(name="ps", bufs=2, space="PSUM"))

        def body(_iv=None):
            bias_sb = bp.tile([128, _G // 128], f32, tag="bias")
            nc.sync.dma_start(out=bias_sb[:], in_=b2[:])

            # Batched loads (all on the SP HWDGE queue; c/outs go on the
            # ScalarE queue inside the group loop). Every tile covers all
            # j/ch so the whole rep's working set streams in up front;
            # bufs=2 lets rep r+1's loads run behind rep r's compute.
            w8_sb, x8_sb, wb_sb, xb_sb = [], [], [], []
            for q in range(_NQ):
                t_ = wp.tile([128, 2, _G // 2], f8, tag=f"w8_{q}")
                nc.sync.dma_start(out=t_[:], in_=wT8[q * 128:(q + 1) * 128, :])
                w8_sb.append(t_)
            for q in range(_NQ):
                t_ = xp.tile([128, 2, _BL], f8, tag=f"x8_{q}")
                nc.sync.dma_start(out=t_[:],
                                  in_=xh8d[q * 128:(q + 1) * 128, :])
                x8_sb.append(t_)
            for k in range(_NK):
                t_ = wp.tile([128, _G // 2], bf16, tag=f"wb_{k}")
                nc.sync.dma_start(out=t_[:], in_=wTb[k * 128:(k + 1) * 128, :])
                wb_sb.append(t_)
            for k in range(_NK):
                t_ = xp.tile([128, _BL], bf16, tag=f"xb_{k}")
                src = xT if k < _NK // 2 else hT
                r = (k % (_NK // 2)) * 128
                nc.sync.dma_start(out=t_[:], in_=src[r:r + 128, :])
                xb_sb.append(t_)

            for ch in range(_NB):
                bsl = slice(ch * _BCH, (ch + 1) * _BCH)
                for j in range(_NJ):
                    c_sb = cp.tile([128, _BCH], bf16, tag="c")
                    nc.gpsimd.dma_start(out=c_sb[:],
                                        in_=cT[j * 128:(j + 1) * 128, bsl])
                    # Issue order interleaves the LDWEIGHTS-bound DoubleRow
                    # matmuls (256-col weight load > their own 107ns stream)
                    # between pairs of stream-bound bf16 matmuls, so every
                    # weight load hides under the previous matmul's stream
                    # via the PE's background weight buffer:
                    #   g_k, o_k, [i|f]_q, g_k+1, o_k+1, ...
                    ps = [pp.tile([128, _BCH], f32, tag=f"ps{t}",
                                  name=f"ps{t}") for t in range(4)]

                    def mm_dr(t, q, start, stop):
                        col = j * 256 + t * 128
                        nc.tensor.matmul(
                            ps[t][:],
                            w8_sb[q][:, :, col:col + 128],
                            x8_sb[q][:, :, bsl],
                            start=start, stop=stop, perf_mode=DR,
                        )

                    def mm_bf(t, k, start, stop):
                        col = j * 256 + (t - 2) * 128
                        nc.tensor.matmul(
                            ps[t][:],
                            wb_sb[k][:, col:col + 128],
                            xb_sb[k][:, bsl],
                            start=start, stop=stop,
                        )

                    for k in range(_NK):
                        mm_bf(2, k, k == 0, k == _NK - 1)
                        mm_bf(3, k, k == 0, k == _NK - 1)
                        t8 = k % 2            # even k -> i, odd k -> f
                        q = k // 2
                        mm_dr(t8, q, q == 0, q == _NQ - 1)
                    gI = ap.tile([128, _BCH], f32, tag="gI")
                    gF = ap.tile([128, _BCH], f32, tag="gF")
                    gG = ap.tile([128, _BCH], f32, tag="gG")
                    gO = ap.tile([128, _BCH], f32, tag="gO")
                    bcol = j * 4
                    nc.scalar.activation(gI[:], ps[0][:], AF.Sigmoid,
                                         bias=bias_sb[:, bcol + 0:bcol + 1],
                                         scale=1.0 / _W8SCALE)
                    nc.scalar.activation(gF[:], ps[1][:], AF.Sigmoid,
                                         bias=bias_sb[:, bcol + 1:bcol + 2],
                                         scale=1.0 / _W8SCALE)
                    nc.scalar.activation(gG[:], ps[2][:], AF.Tanh,
                                         bias=bias_sb[:, bcol + 2:bcol + 3])
                    nc.scalar.activation(gO[:], ps[3][:], AF.Sigmoid,
                                         bias=bias_sb[:, bcol + 3:bcol + 4])
                    newc = op.tile([128, _BCH], bf16, tag="newc")
                    newh = op.tile([128, _BCH], bf16, tag="newh")
                    nc.vector.tensor_mul(gF[:], gF[:], c_sb[:])   # f * c
                    nc.vector.tensor_mul(gI[:], gI[:], gG[:])     # i * g
                    nc.vector.tensor_add(newc[:], gF[:], gI[:])
                    nc.scalar.activation(gG[:], newc[:], AF.Tanh)
                    nc.vector.tensor_mul(newh[:], gO[:], gG[:])
                    nc.gpsimd.dma_start(out=coT[j * 128:(j + 1) * 128, bsl],
                                        in_=newc[:])
                    nc.gpsimd.dma_start(out=hoT[j * 128:(j + 1) * 128, bsl],
                                        in_=newh[:])

        if reps == 1:
            body()
        elif reps < 0:           # python-unrolled (sim analysis only)
            for _ in range(-reps):
                body()
        else:
            with tc.For_i(0, reps, 1):
                body()
    nc.compile()
    return nc


_BF16 = ml_dtypes.bfloat16
_F8 = ml_dtypes.float8_e4m3


def _host_prep_weights(Wi, bi, Wh, bh):
    W = np.concatenate([np.asarray(Wi, np.float32),
                        np.asarray(Wh, np.float32)], axis=1)    # [G, K]
    Wr = W.reshape(4, _NJ, 128, _K)                              # [t, j, m, k]
    # bf16 g/o weights: [k, j*256 + u*128 + m]
    wTb = np.ascontiguousarray(
        Wr[2:].transpose(3, 1, 0, 2).reshape(_K, _G // 2)).astype(_BF16)
    # fp8 i/f weights, DoubleRow layout: [q*128+p, s*1024 + j*256 + t*128 + m]
    W8 = (Wr[:2] * _W8SCALE).astype(_F8)                         # [t, j, m, k]
    W8k = W8.reshape(2, _NJ, 128, _NQ, 2, 128)                   # [t,j,m,q,s,p]
    wT8 = np.ascontiguousarray(
        W8k.transpose(3, 5, 4, 1, 0, 2).reshape(_K // 2, _G))
    # bias, per-partition layout: col = j*4 + t
    b = np.asarray(bi, np.float32) + np.asarray(bh, np.float32)
    br = b.reshape(4, _NJ, 128)                                  # [t, j, p]
    b2 = np.ascontiguousarray(br.transpose(2, 1, 0).reshape(128, _G // 128))
    return wTb, wT8, b2


def _host_shards(x, h, c, Wi, bi, Wh, bh):
    wTb, wT8, b2 = _host_prep_weights(Wi, bi, Wh, bh)
    xf = np.asarray(x, np.float32)
    hf = np.asarray(h, np.float32)
    cf = np.asarray(c, np.float32)
    in_maps = []
    for s in range(_NC):
        sl = slice(s * _BL, (s + 1) * _BL)
        xTs = np.ascontiguousarray(xf[sl].T)                     # [I, BL]
        hTs = np.ascontiguousarray(hf[sl].T)                     # [H, BL]
        xhT = np.concatenate([xTs, hTs], axis=0)                 # [K, BL]
        x8 = xhT.astype(_F8).reshape(_NQ, 2, 128, _BL)           # [q, s, p, b]
        xh8 = np.ascontiguousarray(
            x8.transpose(0, 2, 1, 3).reshape(_K // 2, 2 * _BL))
        in_maps.append({
            "xT": xTs.astype(_BF16),
            "hT": hTs.astype(_BF16),
            "cT": np.ascontiguousarray(cf[sl].T).astype(_BF16),
            "wTb": wTb,
            "wT8": wT8,
            "xh8": xh8,
            "b2": b2,
        })
    return in_maps


def kernel(x, h, c, Wi, bi, Wh, bh):
    from concourse.bass_utils import run_bass_kernel_spmd

    nc = _cache.get("nc")
    if nc is None:
        nc = _build()
        _cache["nc"] = nc

    in_maps = _host_shards(x, h, c, Wi, bi, Wh, bh)
    res = run_bass_kernel_spmd(nc, in_maps, list(range(_NC)))

    h_out = np.empty((_B, _H), np.float32)
    c_out = np.empty((_B, _H), np.float32)
    for s in range(_NC):
        sl = slice(s * _BL, (s + 1) * _BL)
        h_out[sl] = res.results[s]["hoT"].T.astype(np.float32)
        c_out[sl] = res.results[s]["coT"].T.astype(np.float32)
    return h_out, c_out


# revision 29
# speedup vs baseline: 2.3968x; 1.1343x over previous
"""LSTMCell (B=16384, I=H=512) on 8 Trainium2 NeuronCores.

Strategy: data-parallel over the batch (2048 rows/core). Each core computes
gatesT = W @ [x;h]T in transposed layout (gate dim on partitions, batch on the
free dim) so that:
  - the contraction dim (I+H) lands on SBUF partitions for both matmul
    operands with zero on-chip transposes (inputs are pre-transposed on the
    host while sharding),
  - the gate bias is a per-partition vector, applied for free by the ScalarE
    activation instruction (which also folds in the fp8 weight descale).
Mixed precision: the i and f gates run in fp8e4 with perf_mode=DoubleRow
(2 fp8 weights per PE cell, contraction 256/instruction) which cuts their
matmul stream cycles in half; g and o stay bf16 (error analysis: fp8 error
on i+f lands at ~1.7e-2 rel_l2, within the 2e-2 budget, while touching g or
o would exceed it). Weights for the fp8 gates are host-scaled by 64 to avoid
the e4m3 denormal range; ScalarE activation applies scale=1/64.
All wire tensors are bf16/fp8: ~12MB HBM traffic per core vs 28MB for f32.
Elementwise LSTM tail (sigmoid/tanh/mul/add) runs on ScalarE + VectorE
overlapped with the matmuls; outputs are stored transposed in bf16 and
un-transposed/upcast on the host.
"""

import numpy as np
import ml_dtypes
from contextlib import ExitStack

_B, _I, _H = 16384, 512, 512
_NC = 8
_BL = _B // _NC          # 2048 batch rows per core
_G = 4 * _H              # 2048 stacked gate dim
_K = _I + _H             # 1024 contraction dim
_BCH = 512               # batch chunk (PSUM bank free size)
_NB = _BL // _BCH        # 4 batch chunks
_NJ = _H // 128          # 4 h-blocks of 128
_NK = _K // 128          # 8 k-chunks of 128
_NQ = _NK // 2           # 4 DoubleRow k-pairs
_W8SCALE = 64.0

_cache = {}


def _build(reps=1):
    from concourse import bacc
    import concourse.mybir as mybir
    import concourse.tile as tile

    f32 = mybir.dt.float32
    bf16 = mybir.dt.bfloat16
    f8 = mybir.dt.float8e4
    AF = mybir.ActivationFunctionType
    DR = mybir.MatmulPerfMode.DoubleRow

    nc = bacc.Bacc("TRN2", target_bir_lowering=False, debug=False,
                   num_devices=_NC)
    xT = nc.declare_dram_parameter("xT", [_I, _BL], bf16, isOutput=False)
    hT = nc.declare_dram_parameter("hT", [_H, _BL], bf16, isOutput=False)
    cT = nc.declare_dram_parameter("cT", [_H, _BL], bf16, isOutput=False)
    # bf16 weights for the g/o gates: row k, col j*256 + u*128 + m (u: g,o)
    wTb = nc.declare_dram_parameter("wTb", [_K, _G // 2], bf16, isOutput=False)
    # fp8 weights for the i/f gates, DoubleRow-interleaved: row q*128 + p,
    # col s*1024 + j*256 + t*128 + m holds W[t-gate, j, m, k=q*256+s*128+p]*64
    wT8 = nc.declare_dram_parameter("wT8", [_K // 2, _G], f8, isOutput=False)
    # fp8 activations, DoubleRow-interleaved: row q*128 + p,
    # col s*2048 + b holds xh[k=q*256+s*128+p, b]
    xh8d = nc.declare_dram_parameter("xh8", [_K // 2, 2 * _BL], f8,
                                     isOutput=False)
    b2 = nc.declare_dram_parameter("b2", [128, _G // 128], f32, isOutput=False)
    # fused output: [:, 0, :] = new_c^T, [:, 1, :] = new_h^T
    hco = nc.declare_dram_parameter("hco", [_H, 2, _BL], bf16, isOutput=True)

    with ExitStack() as ctx:
        tc = ctx.enter_context(tile.TileContext(nc))
        wp = ctx.enter_context(tc.tile_pool(name="w", bufs=2))
        xp = ctx.enter_context(tc.tile_pool(name="xh", bufs=2))
        bp = ctx.enter_context(tc.tile_pool(name="bias", bufs=1))
        cp = ctx.enter_context(tc.tile_pool(name="cin", bufs=4))
        ap = ctx.enter_context(tc.tile_pool(name="act", bufs=2))
        op = ctx.enter_context(tc.tile_pool(name="out", bufs=3))
        pp = ctx.enter_context(tc.tile_pool(name="ps", bufs=2, space="PSUM"))

        def body(_iv=None):
            bias_sb = bp.tile([128, _G // 128], f32, tag="bias")
            nc.sync.dma_start(out=bias_sb[:], in_=b2[:])

            # Batched loads (all on the SP HWDGE queue; c/outs go on the
            # ScalarE queue inside the group loop). Every tile covers all
            # j/ch so the whole rep's working set streams in up front;
            # bufs=2 lets rep r+1's loads run behind rep r's compute.
            w8_sb, x8_sb, wb_sb, xb_sb = [], [], [], []
            for q in range(_NQ):
                t_ = wp.tile([128, 2, _G // 2], f8, tag=f"w8_{q}")
                nc.sync.dma_start(out=t_[:], in_=wT8[q * 128:(q + 1) * 128, :])
                w8_sb.append(t_)
            for q in range(_NQ):
                t_ = xp.tile([128, 2, _BL], f8, tag=f"x8_{q}")
                nc.sync.dma_start(out=t_[:],
                                  in_=xh8d[q * 128:(q + 1) * 128, :])
                x8_sb.append(t_)
            for k in range(_NK):
                t_ = wp.tile([128, _G // 2], bf16, tag=f"wb_{k}")
                nc.sync.dma_start(out=t_[:], in_=wTb[k * 128:(k + 1) * 128, :])
                wb_sb.append(t_)
            for k in range(_NK):
                t_ = xp.tile([128, _BL], bf16, tag=f"xb_{k}")
                src = xT if k < _NK // 2 else hT
                r = (k % (_NK // 2)) * 128
                nc.sync.dma_start(out=t_[:], in_=src[r:r + 128, :])
                xb_sb.append(t_)

            for ch in range(_NB):
                bsl = slice(ch * _BCH, (ch + 1) * _BCH)
                for j in range(_NJ):
                    c_sb = cp.tile([128, _BCH], bf16, tag="c")
                    nc.gpsimd.dma_start(out=c_sb[:],
                                        in_=cT[j * 128:(j + 1) * 128, bsl])
                    # Issue order interleaves the LDWEIGHTS-bound DoubleRow
                    # matmuls (256-col weight load > their own 107ns stream)
                    # between pairs of stream-bound bf16 matmuls, so every
                    # weight load hides under the previous matmul's stream
                    # via the PE's background weight buffer:
                    #   g_k, o_k, [i|f]_q, g_k+1, o_k+1, ...
                    ps = [pp.tile([128, _BCH], f32, tag=f"ps{t}",
                                  name=f"ps{t}") for t in range(4)]

                    def mm_dr(t, q, start, stop):
                        col = j * 256 + t * 128
                        nc.tensor.matmul(
                            ps[t][:],
                            w8_sb[q][:, :, col:col + 128],
                            x8_sb[q][:, :, bsl],
                            start=start, stop=stop, perf_mode=DR,
                        )

                    def mm_bf(t, k, start, stop):
                        col = j * 256 + (t - 2) * 128
                        nc.tensor.matmul(
                            ps[t][:],
                            wb_sb[k][:, col:col + 128],
                            xb_sb[k][:, bsl],
                            start=start, stop=stop,
                        )

                    for k in range(_NK):
                        mm_bf(2, k, k == 0, k == _NK - 1)
                        mm_bf(3, k, k == 0, k == _NK - 1)
                        t8 = k % 2            # even k -> i, odd k -> f
                        q = k // 2
                        mm_dr(t8, q, q == 0, q == _NQ - 1)
                    gI = ap.tile([128, _BCH], f32, tag="gI")
                    gF = ap.tile([128, _BCH], f32, tag="gF")
                    gG = ap.tile([128, _BCH], f32, tag="gG")
                    gO = ap.tile([128, _BCH], f32, tag="gO")
                    bcol = j * 4
                    nc.scalar.activation(gI[:], ps[0][:], AF.Sigmoid,
                                         bias=bias_sb[:, bcol + 0:bcol + 1],
                                         scale=1.0 / _W8SCALE)
                    nc.scalar.activation(gF[:], ps[1][:], AF.Sigmoid,
                                         bias=bias_sb[:, bcol + 1:bcol + 2],
                                         scale=1.0 / _W8SCALE)
                    nc.scalar.activation(gG[:], ps[2][:], AF.Tanh,
                                         bias=bias_sb[:, bcol + 2:bcol + 3])
                    nc.scalar.activation(gO[:], ps[3][:], AF.Sigmoid,
                                         bias=bias_sb[:, bcol + 3:bcol + 4])
                    hc = op.tile([128, 2 * _BCH], bf16, tag="hc")
                    nc.vector.tensor_mul(gF[:], gF[:], c_sb[:])   # f * c
                    nc.vector.tensor_mul(gI[:], gI[:], gG[:])     # i * g
                    nc.vector.tensor_add(hc[:, 0:_BCH], gF[:], gI[:])
                    nc.scalar.activation(gG[:], hc[:, 0:_BCH], AF.Tanh)
                    nc.vector.tensor_mul(hc[:, _BCH:2 * _BCH], gO[:], gG[:])
                    nc.gpsimd.dma_start(
                        out=hco[j * 128:(j + 1) * 128, :, bsl], in_=hc[:])

        if reps == 1:
            body()
        elif reps < 0:           # python-unrolled (sim analysis only)
            for _ in range(-reps):
                body()
        else:
            with tc.For_i(0, reps, 1):
                body()
    nc.compile()
    return nc


_BF16 = ml_dtypes.bfloat16
_F8 = ml_dtypes.float8_e4m3


def _host_prep_weights(Wi, bi, Wh, bh):
    W = np.concatenate([np.asarray(Wi, np.float32),
                        np.asarray(Wh, np.float32)], axis=1)    # [G, K]
    Wr = W.reshape(4, _NJ, 128, _K)                              # [t, j, m, k]
    # bf16 g/o weights: [k, j*256 + u*128 + m]
    wTb = np.ascontiguousarray(
        Wr[2:].transpose(3, 1, 0, 2).reshape(_K, _G // 2)).astype(_BF16)
    # fp8 i/f weights, DoubleRow layout: [q*128+p, s*1024 + j*256 + t*128 + m]
    W8 = (Wr[:2] * _W8SCALE).astype(_F8)                         # [t, j, m, k]
    W8k = W8.reshape(2, _NJ, 128, _NQ, 2, 128)                   # [t,j,m,q,s,p]
    wT8 = np.ascontiguousarray(
        W8k.transpose(3, 5, 4, 1, 0, 2).reshape(_K // 2, _G))
    # bias, per-partition layout: col = j*4 + t
    b = np.asarray(bi, np.float32) + np.asarray(bh, np.float32)
    br = b.reshape(4, _NJ, 128)                                  # [t, j, p]
    b2 = np.ascontiguousarray(br.transpose(2, 1, 0).reshape(128, _G // 128))
    return wTb, wT8, b2


def _host_shards(x, h, c, Wi, bi, Wh, bh):
    wTb, wT8, b2 = _host_prep_weights(Wi, bi, Wh, bh)
    xf = np.asarray(x, np.float32)
    hf = np.asarray(h, np.float32)
    cf = np.asarray(c, np.float32)
    in_maps = []
    for s in range(_NC):
        sl = slice(s * _BL, (s + 1) * _BL)
        xTs = np.ascontiguousarray(xf[sl].T)                     # [I, BL]
        hTs = np.ascontiguousarray(hf[sl].T)                     # [H, BL]
        xhT = np.concatenate([xTs, hTs], axis=0)                 # [K, BL]
        x8 = xhT.astype(_F8).reshape(_NQ, 2, 128, _BL)           # [q, s, p, b]
        xh8 = np.ascontiguousarray(
            x8.transpose(0, 2, 1, 3).reshape(_K // 2, 2 * _BL))
        in_maps.append({
            "xT": xTs.astype(_BF16),
            "hT": hTs.astype(_BF16),
            "cT": np.ascontiguousarray(cf[sl].T).astype(_BF16),
            "wTb": wTb,
            "wT8": wT8,
            "xh8": xh8,
            "b2": b2,
        })
    return in_maps


def kernel(x, h, c, Wi, bi, Wh, bh):
    from concourse.bass_utils import run_bass_kernel_spmd

    nc = _cache.get("nc")
    if nc is None:
        nc = _build()
        _cache["nc"] = nc

    in_maps = _host_shards(x, h, c, Wi, bi, Wh, bh)
    res = run_bass_kernel_spmd(nc, in_maps, list(range(_NC)))

    h_out = np.empty((_B, _H), np.float32)
    c_out = np.empty((_B, _H), np.float32)
    for s in range(_NC):
        sl = slice(s * _BL, (s + 1) * _BL)
        hco = res.results[s]["hco"]                              # [H, 2, BL]
        c_out[sl] = hco[:, 0, :].T.astype(np.float32)
        h_out[sl] = hco[:, 1, :].T.astype(np.float32)
    return h_out, c_out
